# revision 10
# baseline (speedup 1.0000x reference)
"""PhysNetCore GNN message passing on 8 Trainium2 NeuronCores (Bass/Tile).

Strategy:
- Atoms are sharded round-robin by 128-atom block across 8 cores. Edges are
  sharded by the owner of idx_i, so the scatter-add stays core-local.
- Each core computes q = softplus(pe @ W_j.T + b_j) for its atom shard and
  the row-major q-rows are AllGathered into a replicated full table (the
  "halo gather"); per-edge q_j rows are then fetched with dma_gather
  (indices are int16, so the table is addressed in 4 chunks).
- The scatter-add runs on-chip: per 128-atom block a one-hot matrix (batched
  DVE is_equal) is matmul'd against the messages, accumulating in PSUM.
  (dma_scatter_add has a CCE read-modify-write race on duplicate indices.)
- Dense atom-wise MLP phases run in feature-transposed layout with two
  512-atom tiles stacked into the 128 partitions (block-diagonal weights).
  softplus = Ln(Exp(x)+1) on ACT (one table set); per-feature biases fold
  into the Exp bias (per-partition in transposed layout) or are preloaded
  into PSUM via a rank-1 matmul where the layout is row-major.
"""
import math

import numpy as np
import ml_dtypes

F, R, A = 64, 16, 2
NC = 8


class Cfg:
    def __init__(self, n_atoms, cell_min=640):
        self.N = n_atoms
        # atoms per core: multiple of 1024 (one stacked double-tile = 1024)
        self.LOC = 1024 * math.ceil(n_atoms / (NC * 1024))
        self.NBLK = self.LOC // 128
        self.SUP = self.NBLK // 4
        self.DT = self.NBLK // 8
        self.NPAD = self.LOC * NC
        self.NCHUNK = 4
        self.CHUNK = self.NPAD // self.NCHUNK
        assert self.CHUNK < 32768, "int16 gather index limit"
        assert self.NBLK % 8 == 0
        self.cell_min = cell_min


CFG_FULL = Cfg(100000)


def _wrap_idx(idx):
    """int16 index list [n] -> [128, n/16] wrapped + replicated layout."""
    n = idx.shape[0]
    w = idx.astype(np.int16).reshape(n // 16, 16).T
    return np.tile(w, (8, 1))


def _atom_rows(cfg, core):
    """Global atom ids owned by `core`, in (block, a_loc) order."""
    k = np.arange(cfg.LOC)
    return (k // 128) * (128 * NC) + core * 128 + (k % 128)


def _stack2(w):
    s = np.zeros((128, 128), np.float32)
    s[:64, :64] = w.T
    s[64:, 64:] = w.T
    return s


def _col2(b):
    return np.concatenate([b, b]).astype(np.float32).reshape(128, 1)


# ---------------------------------------------------------------- host prep

def prep_inputs(inputs, cfg=CFG_FULL):
    """Full inputs -> (in_maps per core, CELL, metas for unshard)."""
    pi = np.asarray(inputs["pair_indices"])
    i = pi[0].astype(np.int64)
    j = pi[1].astype(np.int64)
    f_ij = np.asarray(inputs["f_ij"], np.float32)
    emb = np.asarray(inputs["atomic_embedding"], np.float32)
    P = i.shape[0]
    LOC, SUP, NCHUNK, CHUNK, DT = cfg.LOC, cfg.SUP, cfg.NCHUNK, cfg.CHUNK, cfg.DT

    gi = i >> 7
    core = gi % NC
    blkl = gi // NC
    aloc = (i & 127).astype(np.float32)
    gj = j >> 7
    jrow = (gj % NC) * LOC + (gj // NC) * 128 + (j & 127)
    chunk = jrow // CHUNK
    jloc = (jrow % CHUNK).astype(np.int16)
    sup = blkl // 4
    b4 = blkl % 4

    cellid = (((core * SUP + sup) * NCHUNK + chunk) * 4 + b4).astype(np.int64)
    ncells = NC * SUP * NCHUNK * 4
    counts = np.bincount(cellid, minlength=ncells)
    CELL = max(cfg.cell_min, int(math.ceil(counts.max() / 128)) * 128)
    TPC = CELL // 128
    ntiles = SUP * NCHUNK * 4 * TPC

    order = np.argsort(cellid, kind="stable")
    sc = cellid[order]
    starts = np.zeros(ncells + 1, np.int64)
    starts[1:] = np.cumsum(counts)
    pos = np.arange(P) - starts[sc]
    slot = sc * CELL + pos
    nslot = ncells * CELL
    jloc_pad = np.zeros(nslot, np.int16)
    jloc_pad[slot] = jloc[order]
    aloc_pad = np.zeros(nslot, np.float32)
    aloc_pad[slot] = aloc[order]
    fij_pad = np.zeros((nslot, R), np.float32)
    fij_pad[slot] = f_ij[order]

    W_g = np.asarray(inputs["W_g"], np.float32)
    W_i = np.asarray(inputs["W_i"], np.float32)
    W_j = np.asarray(inputs["W_j"], np.float32)
    W_v = np.asarray(inputs["W_v"], np.float32)
    b_i = np.asarray(inputs["b_i"], np.float32)
    b_j = np.asarray(inputs["b_j"], np.float32)
    b_v = np.asarray(inputs["b_v"], np.float32)
    gate = np.asarray(inputs["gate"], np.float32)
    rin_W1 = np.asarray(inputs["rin_W1"], np.float32)
    rin_b1 = np.asarray(inputs["rin_b1"], np.float32)
    rin_W2 = np.asarray(inputs["rin_W2"], np.float32)
    rin_b2 = np.asarray(inputs["rin_b2"], np.float32)
    rout_W1 = np.asarray(inputs["rout_W1"], np.float32)
    rout_b1 = np.asarray(inputs["rout_b1"], np.float32)
    rout_W2 = np.asarray(inputs["rout_W2"], np.float32)
    rout_b2 = np.asarray(inputs["rout_b2"], np.float32)
    W_out = np.asarray(inputs["W_out"], np.float32)
    b_out = np.asarray(inputs["b_out"], np.float32)

    wstacks = np.stack([
        _stack2(W_i),
        _stack2(rin_W1[0]), _stack2(rin_W2[0]),
        _stack2(rin_W1[1]), _stack2(rin_W2[1]),
        _stack2(rin_W1[2]), _stack2(rin_W2[2]),
        _stack2(W_v),
        _stack2(rout_W1[0]), _stack2(rout_W2[0]),
        _stack2(rout_W1[1]), _stack2(rout_W2[1]),
    ])

    z = np.zeros(F, np.float32)
    bcols = np.concatenate([
        _col2(b_i),                                      # 0 vinit
        _col2(z), _col2(rin_b1[0]),                      # 1,2 rin r0
        _col2(rin_b2[0]), _col2(rin_b1[1]),              # 3,4 rin r1
        _col2(rin_b2[0] + rin_b2[1]), _col2(rin_b1[2]),  # 5,6 rin r2
        _col2(rin_b2.sum(0)),                            # 7 sp(v)
        _col2(b_v),                                      # 8 upd out add
        _col2(b_v), _col2(rout_b1[0]),                   # 9,10 rout r0
        _col2(b_v + rout_b2[0]), _col2(rout_b1[1]),      # 11,12 rout r1
        _col2(gate),                                     # 13 gate
    ], axis=1)

    bpfin = b_v + rout_b2.sum(0)
    pred_b = W_out @ bpfin + b_out
    bout_col = np.concatenate([pred_b, pred_b]).astype(np.float32).reshape(4, 1)
    wout_st = np.zeros((128, 4), np.float32)
    wout_st[:64, 0:2] = W_out.T
    wout_st[64:, 2:4] = W_out.T

    iota = np.broadcast_to(
        np.arange(128, dtype=np.float32), (128, 4 * TPC, 128)
    ).astype(ml_dtypes.bfloat16)

    shared = {
        "wgT": np.ascontiguousarray(W_g.T),
        "wjT": np.ascontiguousarray(np.vstack([W_j.T, W_j.T])),
        "bj_row": b_j.reshape(1, F).copy(),
        "wstacks": wstacks,
        "bcols": bcols,
        "wout_st": wout_st,
        "bout_col": bout_col,
        "iota_in": iota,
    }

    in_maps, metas = [], []
    percell = SUP * NCHUNK * 4 * CELL
    for c in range(NC):
        s0 = c * percell
        s1 = s0 + percell
        jl = jloc_pad[s0:s1].reshape(SUP, NCHUNK, 4 * CELL)
        gidx = np.empty((SUP, NCHUNK, 128, (4 * CELL) // 16), np.int16)
        for s in range(SUP):
            for ch in range(NCHUNK):
                gidx[s, ch] = _wrap_idx(jl[s, ch])
        fijT = np.ascontiguousarray(
            fij_pad[s0:s1].reshape(SUP, NCHUNK, 4 * CELL, R).transpose(0, 1, 3, 2))
        sidx = np.ascontiguousarray(aloc_pad[s0:s1].reshape(ntiles, 128).T)

        rows = _atom_rows(cfg, c)
        mask = rows < cfg.N
        emb_core = np.zeros((LOC, F), np.float32)
        emb_core[mask] = emb[rows[mask]]
        emb_stack = np.ascontiguousarray(
            emb_core.reshape(DT, 2, 512, F).transpose(1, 3, 0, 2).reshape(128, DT * 512))

        m = dict(shared)
        m.update({"embStack": emb_stack, "fijT": fijT, "gidx": gidx, "sidx": sidx})
        in_maps.append(m)
        metas.append((rows, mask))
    return in_maps, CELL, metas


def unshard(results, metas, cfg=CFG_FULL):
    prediction = np.zeros((cfg.N, A), np.float32)
    updated = np.zeros((cfg.N, F), np.float32)
    for c in range(NC):
        rows, mask = metas[c]
        updT = results[c]["updT"]
        predT = results[c]["predT"]
        upd_core = updT.reshape(2, 64, cfg.DT, 512).transpose(2, 0, 3, 1).reshape(cfg.LOC, F)
        pred_core = predT.reshape(2, 2, cfg.DT, 512).transpose(2, 0, 3, 1).reshape(cfg.LOC, A)
        updated[rows[mask]] = upd_core[mask]
        prediction[rows[mask]] = pred_core[mask]
    return prediction, updated


# ---------------------------------------------------------------- device program

def build_program(CELL, cfg=CFG_FULL):
    import concourse.bacc as bacc
    import concourse.mybir as mybir
    import concourse.tile as tile

    f32 = mybir.dt.float32
    bf16 = mybir.dt.bfloat16
    i16 = mybir.dt.int16
    AF = mybir.ActivationFunctionType
    ALU = mybir.AluOpType
    LOC, SUP, NCHUNK, CHUNK, DT, NBLK = (
        cfg.LOC, cfg.SUP, cfg.NCHUNK, cfg.CHUNK, cfg.DT, cfg.NBLK)
    TPC = CELL // 128
    NT4 = 4 * TPC
    ntiles = SUP * NCHUNK * NT4

    nc = bacc.Bacc("TRN2", target_bir_lowering=False, debug=False,
                   enable_asserts=False, num_devices=NC)

    embStack = nc.dram_tensor("embStack", [128, DT * 512], f32, kind="ExternalInput")
    fijT_d = nc.dram_tensor("fijT", [SUP, NCHUNK, R, 4 * CELL], f32, kind="ExternalInput")
    gidx_d = nc.dram_tensor("gidx", [SUP, NCHUNK, 128, (4 * CELL) // 16], i16, kind="ExternalInput")
    sidx_d = nc.dram_tensor("sidx", [128, ntiles], f32, kind="ExternalInput")
    iota_d = nc.dram_tensor("iota_in", [128, NT4, 128], bf16, kind="ExternalInput")
    wgT_d = nc.dram_tensor("wgT", [R, F], f32, kind="ExternalInput")
    wjT_d = nc.dram_tensor("wjT", [128, F], f32, kind="ExternalInput")
    bj_d = nc.dram_tensor("bj_row", [1, F], f32, kind="ExternalInput")
    wst_d = nc.dram_tensor("wstacks", [12, 128, 128], f32, kind="ExternalInput")
    bcol_d = nc.dram_tensor("bcols", [128, 14], f32, kind="ExternalInput")
    woutst_d = nc.dram_tensor("wout_st", [128, 4], f32, kind="ExternalInput")
    boutc_d = nc.dram_tensor("bout_col", [4, 1], f32, kind="ExternalInput")
    updT_d = nc.dram_tensor("updT", [128, DT * 512], f32, kind="ExternalOutput")
    predT_d = nc.dram_tensor("predT", [4, DT * 512], f32, kind="ExternalOutput")
    vdbg_d = None
    if getattr(cfg, "debug_v", False):
        vdbg_d = nc.dram_tensor("vdbg", [128, DT * 512], f32, kind="ExternalOutput")

    qloc = nc.dram_tensor("qloc", [LOC, F], f32)
    qfull = nc.dram_tensor("qfull", [cfg.NPAD, F], f32, addr_space="Shared")

    (BI, BX0, BH0, BX1, BH1, BX2, BH2, BV3, BVO,
     BXO0, BHO0, BXO1, BHO1, BGATE) = range(14)
    (WI, W1R0, W2R0, W1R1, W2R1, W1R2, W2R2, WV,
     WO10, WO20, WO11, WO21) = range(12)

    with tile.TileContext(nc) as tc:
        with (
            tc.tile_pool(name="consts", bufs=1) as cp,
            tc.tile_pool(name="resid", bufs=1) as rp,
            tc.tile_pool(name="edge", bufs=3) as ep,
            tc.tile_pool(name="edge2", bufs=2) as ep2,
            tc.tile_pool(name="post", bufs=3) as pp,
            tc.tile_pool(name="psg", bufs=2, space="PSUM") as psg,
            tc.tile_pool(name="psv", bufs=1, space="PSUM") as psvp,
            tc.tile_pool(name="psmm", bufs=2, space="PSUM") as psmm,
        ):
            wg = cp.tile([R, F], f32)
            nc.sync.dma_start(out=wg[:], in_=wgT_d[:, :])
            wj = cp.tile([128, F], f32)
            nc.sync.dma_start(out=wj[:], in_=wjT_d[:, :])
            bjr = cp.tile([1, F], f32)
            nc.sync.dma_start(out=bjr[:], in_=bj_d[:, :])
            wtiles = []
            for k in range(12):
                t = cp.tile([128, 128], f32, tag=f"w{k}")
                nc.sync.dma_start(out=t[:], in_=wst_d[k, :, :])
                wtiles.append(t)
            bcols = cp.tile([128, 14], f32)
            nc.sync.dma_start(out=bcols[:], in_=bcol_d[:, :])
            woutst = cp.tile([128, 4], f32)
            nc.sync.dma_start(out=woutst[:], in_=woutst_d[:, :])
            boutc = cp.tile([4, 1], f32)
            nc.sync.dma_start(out=boutc[:], in_=boutc_d[:, :])
            iota = cp.tile([128, NT4, 128], bf16)
            nc.sync.dma_start(out=iota[:], in_=iota_d[:, :, :])
            sidx = cp.tile([128, ntiles], f32)
            nc.sync.dma_start(out=sidx[:], in_=sidx_d[:, :])
            ones1 = cp.tile([1, 128], f32)
            nc.vector.memset(ones1[:], 1.0)

            peStack = rp.tile([128, DT * 512], f32)
            vinitStack = rp.tile([128, DT * 512], f32)

            def sp_cols(dst_ap, src_ap, bias, shape):
                tE = pp.tile(list(shape), f32, tag="tE")
                nc.scalar.activation(tE[:], src_ap, AF.Exp, bias=bias)
                nc.scalar.activation(dst_ap, tE[:], AF.Ln, bias=1.0)

            # phase 1: pe
            for dt in range(DT):
                cols = slice(dt * 512, (dt + 1) * 512)
                te = pp.tile([128, 512], f32, tag="embt")
                nc.sync.dma_start(out=te[:], in_=embStack[:, cols])
                sp_cols(peStack[:, cols], te[:], 0.0, (128, 512))

            # phase 2a: vinit
            for dt in range(DT):
                cols = slice(dt * 512, (dt + 1) * 512)
                ps = psmm.tile([128, 512], f32, tag="mm")
                nc.tensor.matmul(ps[:], wtiles[WI][:], peStack[:, cols])
                sp_cols(vinitStack[:, cols], ps[:], bcols[:, BI:BI + 1], (128, 512))

            # phase 2b: q rows + AllGather
            for blk in range(NBLK):
                a0 = blk * 128
                dt, within = divmod(a0, 1024)
                half = within // 512
                c0 = dt * 512 + within % 512
                pe_sl = peStack[half * 64:(half + 1) * 64, c0:c0 + 128]
                psq = psmm.tile([128, F], f32, tag="mm")
                nc.tensor.matmul(psq[:], ones1[:], bjr[:], start=True, stop=False)
                nc.tensor.matmul(psq[:], pe_sl, wj[half * 64:(half + 1) * 64, :], start=False, stop=True)
                tq = pp.tile([128, F], f32, tag="tq")
                sp_cols(tq[:], psq[:], 0.0, (128, F))
                nc.sync.dma_start(out=qloc[a0:a0 + 128, :], in_=tq[:])

            nc.gpsimd.collective_compute(
                "AllGather", mybir.AluOpType.bypass,
                replica_groups=[list(range(NC))],
                ins=[qloc.ap().opt()], outs=[qfull.ap().opt()],
            )

            # phases 3+4 per double-tile
            for dt in range(DT):
                cols = slice(dt * 512, (dt + 1) * 512)
                pv_t = [psvp.tile([128, 128], f32, tag=f"pv{b}",
                                   name=f"pv{b}_{dt}")
                        for b in range(4)]
                for s2 in range(2):
                    s = 2 * dt + s2
                    for ch in range(NCHUNK):
                        gi = ep.tile([128, (4 * CELL) // 16], i16, tag="gi")
                        nc.sync.dma_start(out=gi[:], in_=gidx_d[s, ch, :, :])
                        gat = ep.tile([128, NT4, F], f32, tag="gat")
                        for e0 in range(0, 4 * CELL, 1024):
                            n1 = min(1024, 4 * CELL - e0)
                            nc.gpsimd.dma_gather(
                                gat[:, e0 // 128:(e0 + n1) // 128, :],
                                qfull[ch * CHUNK:(ch + 1) * CHUNK, :],
                                gi[:, e0 // 16:(e0 + n1) // 16], n1, n1, F)
                        fij = ep.tile([R, 4 * CELL], f32, tag="fij")
                        nc.sync.dma_start(out=fij[:], in_=fijT_d[s, ch, :, :])
                        msgb = ep2.tile([128, NT4, F], bf16, tag="msgb")
                        for g0 in range(0, NT4, 8):
                            g1 = min(g0 + 8, NT4)
                            gp = psg.tile([128, 8, F], f32, tag="gps")
                            for t in range(g0, g1):
                                nc.tensor.matmul(
                                    gp[:, t - g0, :],
                                    fij[:, t * 128:(t + 1) * 128], wg[:])
                            nc.vector.tensor_mul(
                                msgb[:, g0:g1, :], gat[:, g0:g1, :],
                                gp[:, 0:g1 - g0, :])
                        oh = ep2.tile([128, NT4, 128], bf16, tag="oh")
                        t0 = (s * NCHUNK + ch) * NT4
                        nc.vector.tensor_tensor(
                            oh[:], iota[:],
                            sidx[:, t0:t0 + NT4].unsqueeze(-1).broadcast_to(
                                [128, NT4, 128]),
                            op=ALU.is_equal)
                        for tt in range(NT4):
                            b4 = tt // TPC
                            tic = tt % TPC
                            nc.tensor.matmul(
                                pv_t[b4][s2 * 64:(s2 + 1) * 64, :],
                                msgb[:, tt, :], oh[:, tt, :],
                                start=(ch == 0 and tic == 0),
                                stop=(ch == NCHUNK - 1 and tic == TPC - 1))
                x = pp.tile([128, 512], f32, tag="x")
                for b in range(4):
                    c0 = dt * 512 + b * 128
                    nc.vector.tensor_add(x[:, b * 128:(b + 1) * 128],
                                         vinitStack[:, c0:c0 + 128], pv_t[b][:])
                if vdbg_d is not None:
                    nc.sync.dma_start(out=vdbg_d[:, cols], in_=x[:])

                for (wa, wb, bx, bh) in (
                    (W1R0, W2R0, BX0, BH0),
                    (W1R1, W2R1, BX1, BH1),
                    (W1R2, W2R2, BX2, BH2),
                ):
                    h = pp.tile([128, 512], f32, tag="h")
                    sp_cols(h[:], x[:], bcols[:, bx:bx + 1], (128, 512))
                    ps1 = psmm.tile([128, 512], f32, tag="mm")
                    nc.tensor.matmul(ps1[:], wtiles[wa][:], h[:])
                    h2 = pp.tile([128, 512], f32, tag="h2")
                    sp_cols(h2[:], ps1[:], bcols[:, bh:bh + 1], (128, 512))
                    ps2 = psmm.tile([128, 512], f32, tag="mm")
                    nc.tensor.matmul(ps2[:], wtiles[wb][:], h2[:])
                    nc.vector.tensor_add(x[:], x[:], ps2[:])

                v = pp.tile([128, 512], f32, tag="v")
                sp_cols(v[:], x[:], bcols[:, BV3:BV3 + 1], (128, 512))
                psw = psmm.tile([128, 512], f32, tag="mm")
                nc.tensor.matmul(psw[:], wtiles[WV][:], v[:])
                gp2 = pp.tile([128, 512], f32, tag="gp2")
                nc.scalar.activation(gp2[:], peStack[:, cols], AF.Copy,
                                     scale=bcols[:, BGATE:BGATE + 1])
                xo = pp.tile([128, 512], f32, tag="xo")
                nc.vector.tensor_add(xo[:], gp2[:], psw[:])
                uo = pp.tile([128, 512], f32, tag="uo")
                nc.vector.tensor_scalar(uo[:], xo[:], bcols[:, BVO:BVO + 1], None,
                                        op0=ALU.add)
                nc.sync.dma_start(out=updT_d[:, cols], in_=uo[:])

                for (wa, wb, bx, bh) in (
                    (WO10, WO20, BXO0, BHO0),
                    (WO11, WO21, BXO1, BHO1),
                ):
                    h = pp.tile([128, 512], f32, tag="h")
                    sp_cols(h[:], xo[:], bcols[:, bx:bx + 1], (128, 512))
                    ps1 = psmm.tile([128, 512], f32, tag="mm")
                    nc.tensor.matmul(ps1[:], wtiles[wa][:], h[:])
                    h2 = pp.tile([128, 512], f32, tag="h2")
                    sp_cols(h2[:], ps1[:], bcols[:, bh:bh + 1], (128, 512))
                    ps2 = psmm.tile([128, 512], f32, tag="mm")
                    nc.tensor.matmul(ps2[:], wtiles[wb][:], h2[:])
                    nc.vector.tensor_add(xo[:], xo[:], ps2[:])

                psp = psmm.tile([4, 512], f32, tag="mm")
                nc.tensor.matmul(psp[:], woutst[:], xo[:])
                po = pp.tile([4, 512], f32, tag="po")
                nc.vector.tensor_scalar(po[:], psp[:], boutc[:], None, op0=ALU.add)
                nc.sync.dma_start(out=predT_d[:, cols], in_=po[:])

    nc.compile()
    return nc


# ---------------------------------------------------------------- runner

_CACHE = {}


def get_runner(CELL, cfg=CFG_FULL):
    key = (CELL, cfg.N)
    if key in _CACHE:
        return _CACHE[key]
    import jax
    from jax.sharding import Mesh, PartitionSpec
    try:
        from jax.experimental.shard_map import shard_map
    except ImportError:
        from jax import shard_map
    import concourse.mybir as mybir
    from concourse.bass2jax import (
        _bass_exec_p, partition_id_tensor, install_neuronx_cc_hook)

    nc = build_program(CELL, cfg)
    install_neuronx_cc_hook()
    partition_name = nc.partition_id_tensor.name if nc.partition_id_tensor else None

    in_names, out_names, out_avals, zero_outs = [], [], [], []
    for alloc in nc.m.functions[0].allocations:
        if not isinstance(alloc, mybir.MemoryLocationSet):
            continue
        name = alloc.memorylocations[0].name
        if alloc.kind == "ExternalInput":
            if name != partition_name:
                in_names.append(name)
        elif alloc.kind == "ExternalOutput":
            shape = tuple(alloc.tensor_shape)
            dtype = mybir.dt.np(alloc.dtype)
            out_names.append(name)
            out_avals.append(jax.core.ShapedArray(shape, dtype))
            zero_outs.append(np.zeros(shape, dtype))
    n_params = len(in_names)
    n_outs = len(out_avals)
    all_names = list(in_names) + list(out_names)
    if partition_name is not None:
        all_names.append(partition_name)
    donate = tuple(range(n_params, n_params + n_outs))

    def _body(*args):
        operands = list(args)
        if partition_name is not None:
            operands.append(partition_id_tensor())
        outs = _bass_exec_p.bind(
            *operands,
            out_avals=tuple(out_avals),
            in_names=tuple(all_names),
            out_names=tuple(out_names),
            lowering_input_output_aliases=(),
            sim_require_finite=True,
            sim_require_nnan=True,
            nc=nc,
        )
        return tuple(outs)

    devices = jax.devices()[:NC]
    mesh = Mesh(np.asarray(devices), ("core",))
    in_specs = (PartitionSpec("core"),) * (n_params + n_outs)
    out_specs = (PartitionSpec("core"),) * n_outs
    sharded = jax.jit(
        shard_map(_body, mesh=mesh, in_specs=in_specs, out_specs=out_specs,
                  check_rep=False),
        donate_argnums=donate, keep_unused=True)

    runner = {
        "nc": nc, "sharded": sharded, "in_names": in_names,
        "out_names": out_names, "out_avals": out_avals, "zero_outs": zero_outs,
    }
    _CACHE[key] = runner
    return runner


def run_spmd(runner, in_maps):
    concat_in = [
        np.concatenate([np.asarray(in_maps[c][name]) for c in range(NC)], axis=0)
        for name in runner["in_names"]
    ]
    concat_zeros = [
        np.zeros((NC * z.shape[0], *z.shape[1:]), z.dtype)
        for z in runner["zero_outs"]
    ]
    out_arrs = runner["sharded"](*concat_in, *concat_zeros)
    avals = runner["out_avals"]
    return [
        {name: np.asarray(out_arrs[i]).reshape(NC, *avals[i].shape)[c]
         for i, name in enumerate(runner["out_names"])}
        for c in range(NC)
    ]


def kernel(**inputs):
    in_maps, CELL, metas = prep_inputs(inputs)
    runner = get_runner(CELL)
    results = run_spmd(runner, in_maps)
    return unshard(results, metas)
